# revision 11
# baseline (speedup 1.0000x reference)
"""Causal self-attention (B=4, T=2048, C=2048, H=16) on 8 trn2 NeuronCores.

Sharding: tensor-parallel over heads — 2 heads per core. Every core gets the
full (pre-transposed) activation xT, its 2 heads' slice of Wqkv columns and
Wproj rows, computes a full [B*T, C] partial output, and the host sums the 8
partials (the "all-reduce after output projection" done host-side).

v2 dataflow (all matmuls fp16 on PE; softmax tail is PE-free):
  xT tiles --DMA--> QKV proj -> Q^T,K^T [d,t] + V [t,d] (fp16)
  S^T = K^T-block.T @ Q^T chunks (PSUM f32) -> +causal mask (DVE)
  exp (ACT) -> P^T (fp16) -> DVE-accumulated denominator (acc over k-blocks)
  y^T = sum_k V_k^T-block @ P^T-block (PSUM f32)
  den all-partition sum via gpsimd.partition_all_reduce -> DVE fast reciprocal
  yt = y^T * (1/den)  (DVE) -> proj: out_partial = yt.T @ Wproj-rows -> DMA

Scheduling: the PE executes its queue in order, so the emission order IS the
schedule.  Each (b, qg) block interleaves the serial attention steps (score ->
exp -> yt, latency-bound on ACT) with independent "filler" matmul quanta: the
next t-chunk's QKV projection and the previous block's output projection.
The softmax tail (all_reduce + reciprocal + normalize) runs entirely on
gpsimd/DVE, so it never stalls the PE queue.
"""
import numpy as np

B, T, C = 4, 2048, 2048
H, HD = 16, 128
N_CORES = 8
HPC = H // N_CORES          # heads per core = 2
SCALE = float(1.0 / np.sqrt(HD))
NEG = -1e9
MM_DT = "fp16"

_CACHE = {}


def _build_nc():
    import concourse.bass as bass
    from concourse import bacc
    import concourse.tile as tile
    import concourse.mybir as mybir
    import concourse.bass_isa as bass_isa
    from concourse.masks import make_identity
    from contextlib import ExitStack

    f32 = mybir.dt.float32
    f16 = mybir.dt.float16
    Exp = mybir.ActivationFunctionType.Exp

    nc = bacc.Bacc("TRN2", target_bir_lowering=False, debug=False,
                   enable_asserts=True, num_devices=N_CORES)

    xT = nc.dram_tensor("xt", [C, B * T], f16, kind="ExternalInput").ap()
    wqkv = nc.dram_tensor("wqkv", [C, 6 * HD], f16, kind="ExternalInput").ap()
    wproj = nc.dram_tensor("wproj", [HPC * HD, C], f16, kind="ExternalInput").ap()
    out = nc.dram_tensor("out", [B * T, C], f32, kind="ExternalOutput").ap()

    wqkv_v = wqkv.rearrange("(cc p) (jj d) -> p cc jj d", p=128, d=HD)  # [128,16,6,128]
    wproj_v = wproj.rearrange("(jh p) c -> p jh c", p=128)              # [128,2,2048]
    xT_v = xT.rearrange("(cc p) t -> p cc t", p=128)                    # [128,16,B*T]

    NCC = C // 128        # 16 contraction chunks
    NTCH = T // 512       # 4 t-chunks per batch

    with tile.TileContext(nc) as tc, ExitStack() as ctx:
        const = ctx.enter_context(tc.tile_pool(name="const", bufs=1))
        wpool = ctx.enter_context(tc.tile_pool(name="w", bufs=1))
        xtp = ctx.enter_context(tc.tile_pool(name="xt", bufs=2))
        qkvp = ctx.enter_context(tc.tile_pool(name="qkv", bufs=2))
        ptp = ctx.enter_context(tc.tile_pool(name="pt", bufs=6))
        accp = ctx.enter_context(tc.tile_pool(name="acc", bufs=2))
        rp = ctx.enter_context(tc.tile_pool(name="r", bufs=4))
        ysbp = ctx.enter_context(tc.tile_pool(name="ysb", bufs=3))
        ytp = ctx.enter_context(tc.tile_pool(name="yt", bufs=3))
        op = ctx.enter_context(tc.tile_pool(name="o", bufs=6))
        psA = ctx.enter_context(tc.tile_pool(name="psA", bufs=3, space="PSUM"))
        psS = ctx.enter_context(tc.tile_pool(name="psS", bufs=3, space="PSUM"))
        psV = ctx.enter_context(tc.tile_pool(name="psV", bufs=2, space="PSUM"))

        ident_f = const.tile([128, 128], f32)
        make_identity(nc, ident_f)
        ident_h = const.tile([128, 128], f16)
        nc.scalar.copy(ident_h, ident_f)
        # transposed-orientation causal mask: keep (partition=k_rel) <= (free=q_rel)
        triT = const.tile([128, 128], f32)
        nc.gpsimd.memset(triT, 0.0)
        nc.gpsimd.affine_select(
            out=triT, in_=triT, compare_op=mybir.AluOpType.is_ge, fill=NEG,
            base=0, pattern=[[1, 128]], channel_multiplier=-1)
        ones_col = const.tile([128, 1], f16)
        nc.vector.memset(ones_col, 1.0)
        ones_row = const.tile([1, 128], f16)
        nc.vector.memset(ones_row, 1.0)

        w_sb = wpool.tile([128, NCC, 6, HD], f16)
        wp_sb = wpool.tile([128, 2, C], f16)

        def chunk_quanta(b, tch, tiles, split_dma=False):
            """QKV projection of one 512-token chunk, as a list of small
            emission quanta (fillers for the PE between attention steps)."""
            qt, kt, vt, v = tiles
            t0 = b * T + tch * 512
            xt_t = xtp.tile([128, NCC, 512], f16, tag="xt")
            quanta = []

            if split_dma:
                def dma_part(g):
                    def run():
                        nc.sync.dma_start(xt_t[:, 4 * g:4 * g + 4],
                                          xT_v[:, 4 * g:4 * g + 4, t0:t0 + 512])
                    return run
                for g in range(4):
                    quanta.append(dma_part(g))
            else:
                def dma_thunk():
                    nc.sync.dma_start(xt_t, xT_v[:, :, t0:t0 + 512])
                quanta.append(dma_thunk)

            state = {}

            def mm_thunk(jj, ccg):
                def run():
                    if ccg == 0:
                        state[jj] = psA.tile([128, 512], f32, tag="psA", name="qkps")
                    qk_ps = state[jj]
                    for cc in range(4 * ccg, 4 * ccg + 4):
                        nc.tensor.matmul(qk_ps, w_sb[:, cc, jj, :],
                                         xt_t[:, cc, :],
                                         start=(cc == 0), stop=(cc == NCC - 1))
                    if ccg == 3:
                        dst = (qt, qt, kt, kt, vt, vt)[jj]
                        nc.scalar.copy(
                            dst[:, jj % 2, tch * 512:(tch + 1) * 512], qk_ps)
                return run
            for jj in range(6):
                for ccg in range(4):
                    quanta.append(mm_thunk(jj, ccg))

            def tr_thunk(hh, tb):
                def run():
                    tg = tch * 4 + tb
                    vp = psA.tile([128, 128], f16, tag="psA")
                    nc.tensor.transpose(
                        vp, vt[:, hh, tg * 128:(tg + 1) * 128], ident_h)
                    nc.vector.tensor_copy(v[:, tg, hh * HD:(hh + 1) * HD], vp)
                return run
            for hh in range(HPC):
                for tb in range(4):
                    quanta.append(tr_thunk(hh, tb))
            return quanta

        def proj_quanta(b, qg, yt):
            quanta = []

            def pr_thunk(tt, co):
                def run():
                    o_ps = psA.tile([128, 512], f32, tag="psA")
                    for jh in range(HPC):
                        nc.tensor.matmul(
                            o_ps, yt[:, jh, tt * 128:(tt + 1) * 128],
                            wp_sb[:, jh, co * 512:(co + 1) * 512],
                            start=(jh == 0), stop=(jh == HPC - 1))
                    o_sb = op.tile([128, 512], f32, tag="osb")
                    if (tt + co) % 2:  # balance PSUM->SBUF copies across engines
                        nc.scalar.copy(o_sb, o_ps)
                    else:
                        nc.vector.tensor_copy(o_sb, o_ps)
                    r0 = b * T + qg * 512 + tt * 128
                    nc.sync.dma_start(
                        out[r0:r0 + 128, co * 512:(co + 1) * 512], o_sb)
                return run
            for tt in range(4):
                for co in range(4):
                    quanta.append(pr_thunk(tt, co))
            return quanta

        def unit_steps(b, qg, tiles, yt, last=False):
            """Attention for both heads of one 512-query group, h-interleaved.
            Softmax denominator accumulates on DVE; tail is PE-free except in
            the very last block, where a PE den-reduce + broadcast has lower
            latency than the serialized gpsimd all-reduces."""
            qt, kt, vt, v = tiles
            nkb = 4 * qg + 4
            acc = [None, None]
            yt_ps = [None, None]
            ysb = [None, None]

            def step(h, kb):
                def run():
                    kk = kb - 4 * qg
                    qs = max(0, kk) * 128
                    q0 = qg * 512
                    st = psS.tile([128, 512], f32, tag="st")
                    nc.tensor.matmul(
                        st[:, qs:512], kt[:, h, kb * 128:(kb + 1) * 128],
                        qt[:, h, q0 + qs:q0 + 512], start=True, stop=True)
                    if kk >= 0:
                        nc.vector.tensor_add(
                            st[:, qs:qs + 128], st[:, qs:qs + 128], triT)
                    pt = ptp.tile([128, 512], f16, tag="pt")
                    nc.scalar.activation(
                        pt[:, qs:512], st[:, qs:512], Exp, scale=SCALE)
                    if kb == 0:
                        acc[h] = accp.tile([128, 512], f16, tag="acc", name="accd")
                        yt_ps[h] = psV.tile([128, 512], f32, tag="psV", name="ytps")
                        nc.vector.tensor_copy(acc[h], pt)
                    else:
                        nc.vector.tensor_add(
                            acc[h][:, qs:512], acc[h][:, qs:512], pt[:, qs:512])
                    nc.tensor.matmul(
                        yt_ps[h][:, qs:512], v[:, kb, h * HD:(h + 1) * HD],
                        pt[:, qs:512], start=(kb == 0), stop=(kb == nkb - 1))
                return run

            def ycopy(h):
                def run():  # frees the psV bank without waiting on the tail
                    ysb[h] = ysbp.tile([128, 512], f32, tag="ysb", name="ysb")
                    nc.vector.tensor_copy(ysb[h], yt_ps[h])
                return run

            def tail(h):
                def run():
                    r_all = rp.tile([128, 512], f32, tag="r")
                    nc.gpsimd.partition_all_reduce(
                        r_all, acc[h], 128, bass_isa.ReduceOp.add)
                    rec = rp.tile([128, 512], f32, tag="rec")
                    nc.vector.reciprocal_approx_fast(rec, r_all)
                    nc.vector.tensor_mul(yt[:, h, :], ysb[h], rec)
                return run

            def tail_pe(h):
                def run():
                    den_ps = psA.tile([1, 512], f32, tag="psA", name="denp")
                    nc.tensor.matmul(den_ps, ones_col, acc[h],
                                     start=True, stop=True)
                    rec1 = rp.tile([1, 512], f32, tag="rec")
                    nc.vector.reciprocal_approx_fast(rec1, den_ps)
                    rec16 = rp.tile([1, 512], f16, tag="r")
                    nc.scalar.copy(rec16, rec1)
                    r_ps = psA.tile([128, 512], f32, tag="psA", name="rps")
                    nc.tensor.matmul(r_ps, ones_row, rec16,
                                     start=True, stop=True)
                    rsb = rp.tile([128, 512], f32, tag="rsb")
                    nc.vector.tensor_copy(rsb, r_ps)
                    nc.vector.tensor_mul(yt[:, h, :], ysb[h], rsb)
                return run

            steps = []
            for kb in range(nkb):
                for h in range(HPC):
                    steps.append(step(h, kb))
            steps.append(ycopy(0))
            steps.append(ycopy(1))
            t = tail_pe if last else tail
            steps.append(t(0))
            steps.append(t(1))
            return steps

        def alloc_qkv_tiles():
            qt = qkvp.tile([128, HPC, T], f16, tag="qt")
            kt = qkvp.tile([128, HPC, T], f16, tag="kt")
            vt = qkvp.tile([128, HPC, T], f16, tag="vt")
            v = qkvp.tile([128, T // 128, HPC * HD], f16, tag="v")
            return (qt, kt, vt, v)

        def warmup(tiles):
            """Chunk (0,0) QKV with ccg-outer iteration so each 4-cc group of
            matmuls needs only the w/xt DMA parts already delivered.  The 6 jj
            PSUM accumulators live across ccg groups, spread over 3 pools."""
            qt, kt, vt, v = tiles
            xt_t = xtp.tile([128, NCC, 512], f16, tag="xt")
            pools6 = [(psA, "psA"), (psA, "psA"), (psS, "st"),
                      (psS, "st"), (psV, "psV"), (psV, "psV")]
            nc.sync.dma_start(w_sb[:, 0:4], wqkv_v[:, 0:4])
            nc.sync.dma_start(xt_t[:, 0:4], xT_v[:, 0:4, 0:512])
            state = {}
            for g in range(4):
                if g < 3:
                    s = 4 * (g + 1)
                    nc.sync.dma_start(w_sb[:, s:s + 4], wqkv_v[:, s:s + 4])
                    nc.sync.dma_start(xt_t[:, s:s + 4], xT_v[:, s:s + 4, 0:512])
                for jj in range(6):
                    if g == 0:
                        pool, tag = pools6[jj]
                        state[jj] = pool.tile([128, 512], f32, tag=tag,
                                              name="wmps")
                    for cc in range(4 * g, 4 * g + 4):
                        nc.tensor.matmul(state[jj], w_sb[:, cc, jj, :],
                                         xt_t[:, cc, :],
                                         start=(cc == 0), stop=(cc == NCC - 1))
                    if g == 3:
                        dst = (qt, qt, kt, kt, vt, vt)[jj]
                        nc.scalar.copy(dst[:, jj % 2, 0:512], state[jj])
            nc.sync.dma_start(wp_sb, wproj_v)
            for hh in range(HPC):
                for tb in range(4):
                    vp = psA.tile([128, 128], f16, tag="psA")
                    nc.tensor.transpose(vp, vt[:, hh, tb * 128:(tb + 1) * 128],
                                        ident_h)
                    nc.vector.tensor_copy(v[:, tb, hh * HD:(hh + 1) * HD], vp)

        chunks = [(b, t) for b in range(B) for t in range(NTCH)]
        tiles_cur = alloc_qkv_tiles()
        tiles_nxt = None
        warmup(tiles_cur)
        ci = 1
        pending = []
        for b in range(B):
            for qg in range(NTCH):
                last = (b == B - 1 and qg == NTCH - 1)
                fillers = []
                if ci < len(chunks):
                    cb, ct = chunks[ci]
                    ci += 1
                    if cb != b:
                        tiles_nxt = alloc_qkv_tiles()
                    fillers += chunk_quanta(
                        cb, ct, tiles_cur if cb == b else tiles_nxt)
                if len(pending) >= 2:  # proj deferred two blocks: its yt
                    fillers += proj_quanta(*pending.pop(0))  # is long ready
                yt = ytp.tile([128, HPC, 512], f16, tag="yt")
                steps = unit_steps(b, qg, tiles_cur, yt, last=last)
                nf, ns = len(fillers), len(steps)
                fi = min(2, nf)
                for q in fillers[:fi]:  # prime the PE (incl. the xt DMA)
                    q()
                for si, s in enumerate(steps):
                    s()
                    tgt = min(2 + (si + 1) * (nf - 2) // ns, nf) if nf > 2 else fi
                    while fi < tgt:
                        fillers[fi]()
                        fi += 1
                while fi < nf:
                    fillers[fi]()
                    fi += 1
                pending.append((b, qg, yt))
            if tiles_nxt is not None:
                tiles_cur, tiles_nxt = tiles_nxt, None
        for pp in pending:  # (3,2) covers the last block's tail; (3,3) drains
            for q in proj_quanta(*pp):
                q()

    nc.compile()
    return nc


def _get_nc():
    if "nc" not in _CACHE:
        _CACHE["nc"] = _build_nc()
    return _CACHE["nc"]


def _make_in_maps(x2d, Wqkv, Wproj):
    hdt = np.float16
    xT = np.ascontiguousarray(x2d.T).astype(hdt)  # [C, B*T]
    in_maps = []
    for c in range(N_CORES):
        h0 = c * HPC
        cols = []
        for part in range(3):  # q, k, v blocks of Wqkv columns
            for h in range(HPC):
                j0 = part * C + (h0 + h) * HD
                cols.append(Wqkv[:, j0:j0 + HD])
        wq = np.ascontiguousarray(np.concatenate(cols, axis=1)).astype(hdt)
        wp = np.ascontiguousarray(Wproj[h0 * HD:(h0 + HPC) * HD, :]).astype(hdt)
        in_maps.append({"xt": xT, "wqkv": wq, "wproj": wp})
    return in_maps


def run_shards(in_maps, trace=False):
    from concourse.bass_utils import run_bass_kernel_spmd
    nc = _get_nc()
    last_err = None
    for _attempt in range(3):
        try:
            return run_bass_kernel_spmd(
                nc, in_maps, core_ids=list(range(N_CORES)), trace=trace)
        except Exception as e:  # transient NRT device errors — retry
            last_err = e
            if "UNAVAILABLE" not in str(e) and "UNRECOVERABLE" not in str(e):
                raise
    raise last_err


def kernel(x, Wqkv, Wproj):
    x = np.asarray(x, dtype=np.float32)
    Wqkv = np.asarray(Wqkv, dtype=np.float32)
    Wproj = np.asarray(Wproj, dtype=np.float32)
    x2d = np.ascontiguousarray(x.reshape(B * T, C))

    in_maps = _make_in_maps(x2d, Wqkv, Wproj)
    res = run_shards(in_maps)

    acc = res.results[0]["out"].astype(np.float64)
    for c in range(1, N_CORES):
        acc += res.results[c]["out"]
    return acc.reshape(B, T, C).astype(np.float32)


# revision 12
# speedup vs baseline: 1.0128x; 1.0128x over previous
"""Causal self-attention (B=4, T=2048, C=2048, H=16) on 8 trn2 NeuronCores.

Sharding: tensor-parallel over heads — 2 heads per core. Every core gets the
full (pre-transposed) activation xT, its 2 heads' slice of Wqkv columns and
Wproj rows, computes a full [B*T, C] partial output, and the host sums the 8
partials (the "all-reduce after output projection" done host-side).

v2 dataflow (all matmuls fp16 on PE; softmax tail is PE-free):
  xT tiles --DMA--> QKV proj -> Q^T,K^T [d,t] + V [t,d] (fp16)
  S^T = K^T-block.T @ Q^T chunks (PSUM f32) -> +causal mask (DVE)
  exp (ACT) -> P^T (fp16) -> DVE-accumulated denominator (acc over k-blocks)
  y^T = sum_k V_k^T-block @ P^T-block (PSUM f32)
  den all-partition sum via gpsimd.partition_all_reduce -> DVE fast reciprocal
  yt = y^T * (1/den)  (DVE) -> proj: out_partial = yt.T @ Wproj-rows -> DMA

Scheduling: the PE executes its queue in order, so the emission order IS the
schedule.  Each (b, qg) block interleaves the serial attention steps (score ->
exp -> yt, latency-bound on ACT) with independent "filler" matmul quanta: the
next t-chunk's QKV projection and the previous block's output projection.
The softmax tail (all_reduce + reciprocal + normalize) runs entirely on
gpsimd/DVE, so it never stalls the PE queue.
"""
import numpy as np

B, T, C = 4, 2048, 2048
H, HD = 16, 128
N_CORES = 8
HPC = H // N_CORES          # heads per core = 2
SCALE = float(1.0 / np.sqrt(HD))
NEG = -1e9
MM_DT = "fp16"

_CACHE = {}


def _build_nc():
    import concourse.bass as bass
    from concourse import bacc
    import concourse.tile as tile
    import concourse.mybir as mybir
    import concourse.bass_isa as bass_isa
    from concourse.masks import make_identity
    from contextlib import ExitStack

    f32 = mybir.dt.float32
    f16 = mybir.dt.float16
    Exp = mybir.ActivationFunctionType.Exp

    nc = bacc.Bacc("TRN2", target_bir_lowering=False, debug=False,
                   enable_asserts=True, num_devices=N_CORES)

    xT = nc.dram_tensor("xt", [C, B * T], f16, kind="ExternalInput").ap()
    wqkv = nc.dram_tensor("wqkv", [C, 6 * HD], f16, kind="ExternalInput").ap()
    wproj = nc.dram_tensor("wproj", [HPC * HD, C], f16, kind="ExternalInput").ap()
    out = nc.dram_tensor("out", [B * T, C], f32, kind="ExternalOutput").ap()

    wqkv_v = wqkv.rearrange("(cc p) (jj d) -> p cc jj d", p=128, d=HD)  # [128,16,6,128]
    wproj_v = wproj.rearrange("(jh p) c -> p jh c", p=128)              # [128,2,2048]
    xT_v = xT.rearrange("(cc p) t -> p cc t", p=128)                    # [128,16,B*T]

    NCC = C // 128        # 16 contraction chunks
    NTCH = T // 512       # 4 t-chunks per batch

    with tile.TileContext(nc) as tc, ExitStack() as ctx:
        const = ctx.enter_context(tc.tile_pool(name="const", bufs=1))
        wpool = ctx.enter_context(tc.tile_pool(name="w", bufs=1))
        xtp = ctx.enter_context(tc.tile_pool(name="xt", bufs=2))
        qkvp = ctx.enter_context(tc.tile_pool(name="qkv", bufs=2))
        ptp = ctx.enter_context(tc.tile_pool(name="pt", bufs=6))
        accp = ctx.enter_context(tc.tile_pool(name="acc", bufs=4))
        rp = ctx.enter_context(tc.tile_pool(name="r", bufs=6))
        ysbp = ctx.enter_context(tc.tile_pool(name="ysb", bufs=4))
        ytp = ctx.enter_context(tc.tile_pool(name="yt", bufs=3))
        op = ctx.enter_context(tc.tile_pool(name="o", bufs=6))
        psA = ctx.enter_context(tc.tile_pool(name="psA", bufs=3, space="PSUM"))
        psS = ctx.enter_context(tc.tile_pool(name="psS", bufs=3, space="PSUM"))
        psV = ctx.enter_context(tc.tile_pool(name="psV", bufs=2, space="PSUM"))

        ident_f = const.tile([128, 128], f32)
        make_identity(nc, ident_f)
        ident_h = const.tile([128, 128], f16)
        nc.scalar.copy(ident_h, ident_f)
        # transposed-orientation causal mask: keep (partition=k_rel) <= (free=q_rel)
        triT = const.tile([128, 128], f32)
        nc.gpsimd.memset(triT, 0.0)
        nc.gpsimd.affine_select(
            out=triT, in_=triT, compare_op=mybir.AluOpType.is_ge, fill=NEG,
            base=0, pattern=[[1, 128]], channel_multiplier=-1)
        ones_col = const.tile([128, 1], f16)
        nc.vector.memset(ones_col, 1.0)
        ones_row = const.tile([1, 128], f16)
        nc.vector.memset(ones_row, 1.0)

        w_sb = wpool.tile([128, NCC, 6, HD], f16)
        wp_sb = wpool.tile([128, 2, C], f16)

        def chunk_quanta(b, tch, tiles, split_dma=False):
            """QKV projection of one 512-token chunk, as a list of small
            emission quanta (fillers for the PE between attention steps)."""
            qt, kt, vt, v = tiles
            t0 = b * T + tch * 512
            xt_t = xtp.tile([128, NCC, 512], f16, tag="xt")
            quanta = []

            if split_dma:
                def dma_part(g):
                    def run():
                        nc.sync.dma_start(xt_t[:, 4 * g:4 * g + 4],
                                          xT_v[:, 4 * g:4 * g + 4, t0:t0 + 512])
                    return run
                for g in range(4):
                    quanta.append(dma_part(g))
            else:
                def dma_thunk():
                    nc.sync.dma_start(xt_t, xT_v[:, :, t0:t0 + 512])
                quanta.append(dma_thunk)

            state = {}

            def mm_thunk(jj, ccg):
                def run():
                    if ccg == 0:
                        state[jj] = psA.tile([128, 512], f32, tag="psA", name="qkps")
                    qk_ps = state[jj]
                    for cc in range(4 * ccg, 4 * ccg + 4):
                        nc.tensor.matmul(qk_ps, w_sb[:, cc, jj, :],
                                         xt_t[:, cc, :],
                                         start=(cc == 0), stop=(cc == NCC - 1))
                    if ccg == 3:
                        dst = (qt, qt, kt, kt, vt, vt)[jj]
                        nc.scalar.copy(
                            dst[:, jj % 2, tch * 512:(tch + 1) * 512], qk_ps)
                return run
            for jj in range(6):
                for ccg in range(4):
                    quanta.append(mm_thunk(jj, ccg))

            def tr_thunk(hh, tb):
                def run():
                    tg = tch * 4 + tb
                    vp = psA.tile([128, 128], f16, tag="psA")
                    nc.tensor.transpose(
                        vp, vt[:, hh, tg * 128:(tg + 1) * 128], ident_h)
                    nc.vector.tensor_copy(v[:, tg, hh * HD:(hh + 1) * HD], vp)
                return run
            for hh in range(HPC):
                for tb in range(4):
                    quanta.append(tr_thunk(hh, tb))
            return quanta

        def proj_quanta(b, qg, yt):
            quanta = []

            def pr_thunk(tt, co):
                def run():
                    o_ps = psA.tile([128, 512], f32, tag="psA")
                    for jh in range(HPC):
                        nc.tensor.matmul(
                            o_ps, yt[:, jh, tt * 128:(tt + 1) * 128],
                            wp_sb[:, jh, co * 512:(co + 1) * 512],
                            start=(jh == 0), stop=(jh == HPC - 1))
                    o_sb = op.tile([128, 512], f32, tag="osb")
                    nc.vector.tensor_copy(o_sb, o_ps)
                    r0 = b * T + qg * 512 + tt * 128
                    nc.sync.dma_start(
                        out[r0:r0 + 128, co * 512:(co + 1) * 512], o_sb)
                return run
            for tt in range(4):
                for co in range(4):
                    quanta.append(pr_thunk(tt, co))
            return quanta

        def unit_steps(b, qg, tiles, yt, last=False):
            """Attention for both heads of one 512-query group, h-interleaved.
            Softmax denominator accumulates on DVE; tail is PE-free except in
            the very last block, where a PE den-reduce + broadcast has lower
            latency than the serialized gpsimd all-reduces."""
            qt, kt, vt, v = tiles
            nkb = 4 * qg + 4
            acc = [None, None]
            yt_ps = [None, None]
            ysb = [None, None]

            def step(h, kb):
                def run():
                    kk = kb - 4 * qg
                    qs = max(0, kk) * 128
                    q0 = qg * 512
                    st = psS.tile([128, 512], f32, tag="st")
                    nc.tensor.matmul(
                        st[:, qs:512], kt[:, h, kb * 128:(kb + 1) * 128],
                        qt[:, h, q0 + qs:q0 + 512], start=True, stop=True)
                    if kk >= 0:
                        nc.vector.tensor_add(
                            st[:, qs:qs + 128], st[:, qs:qs + 128], triT)
                    pt = ptp.tile([128, 512], f16, tag="pt")
                    nc.scalar.activation(
                        pt[:, qs:512], st[:, qs:512], Exp, scale=SCALE)
                    if kb == 0:
                        acc[h] = accp.tile([128, 512], f16, tag="acc", name="accd")
                        yt_ps[h] = psV.tile([128, 512], f32, tag="psV", name="ytps")
                        nc.vector.tensor_copy(acc[h], pt)
                    else:
                        nc.vector.tensor_add(
                            acc[h][:, qs:512], acc[h][:, qs:512], pt[:, qs:512])
                    nc.tensor.matmul(
                        yt_ps[h][:, qs:512], v[:, kb, h * HD:(h + 1) * HD],
                        pt[:, qs:512], start=(kb == 0), stop=(kb == nkb - 1))
                return run

            def ycopy(h):
                def run():  # frees the psV bank without waiting on the tail
                    ysb[h] = ysbp.tile([128, 512], f32, tag="ysb", name="ysb")
                    nc.vector.tensor_copy(ysb[h], yt_ps[h])
                return run

            def tail(h):
                def run():
                    r_all = rp.tile([128, 512], f32, tag="r")
                    nc.gpsimd.partition_all_reduce(
                        r_all, acc[h], 128, bass_isa.ReduceOp.add)
                    rec = rp.tile([128, 512], f32, tag="rec")
                    nc.vector.reciprocal_approx_fast(rec, r_all)
                    nc.vector.tensor_mul(yt[:, h, :], ysb[h], rec)
                return run

            def tail_pe(h):
                def run():
                    den_ps = psA.tile([1, 512], f32, tag="psA", name="denp")
                    nc.tensor.matmul(den_ps, ones_col, acc[h],
                                     start=True, stop=True)
                    rec1 = rp.tile([1, 512], f32, tag="rec")
                    nc.vector.reciprocal_approx_fast(rec1, den_ps)
                    rec16 = rp.tile([1, 512], f16, tag="r")
                    nc.scalar.copy(rec16, rec1)
                    r_ps = psA.tile([128, 512], f32, tag="psA", name="rps")
                    nc.tensor.matmul(r_ps, ones_row, rec16,
                                     start=True, stop=True)
                    rsb = rp.tile([128, 512], f32, tag="rsb")
                    nc.vector.tensor_copy(rsb, r_ps)
                    nc.vector.tensor_mul(yt[:, h, :], ysb[h], rsb)
                return run

            steps = []
            for kb in range(nkb):
                for h in range(HPC):
                    steps.append(step(h, kb))
            steps.append(ycopy(0))
            steps.append(ycopy(1))
            t = tail_pe if last else tail
            steps.append(t(0))
            steps.append(t(1))
            return steps

        def alloc_qkv_tiles():
            qt = qkvp.tile([128, HPC, T], f16, tag="qt")
            kt = qkvp.tile([128, HPC, T], f16, tag="kt")
            vt = qkvp.tile([128, HPC, T], f16, tag="vt")
            v = qkvp.tile([128, T // 128, HPC * HD], f16, tag="v")
            return (qt, kt, vt, v)

        def warmup(tiles):
            """Chunk (0,0) QKV with ccg-outer iteration so each 4-cc group of
            matmuls needs only the w/xt DMA parts already delivered.  The 6 jj
            PSUM accumulators live across ccg groups, spread over 3 pools."""
            qt, kt, vt, v = tiles
            xt_t = xtp.tile([128, NCC, 512], f16, tag="xt")
            pools6 = [(psA, "psA"), (psA, "psA"), (psS, "st"),
                      (psS, "st"), (psV, "psV"), (psV, "psV")]
            nc.sync.dma_start(w_sb[:, 0:4], wqkv_v[:, 0:4])
            nc.sync.dma_start(xt_t[:, 0:4], xT_v[:, 0:4, 0:512])
            state = {}
            for g in range(4):
                if g < 3:
                    s = 4 * (g + 1)
                    nc.sync.dma_start(w_sb[:, s:s + 4], wqkv_v[:, s:s + 4])
                    nc.sync.dma_start(xt_t[:, s:s + 4], xT_v[:, s:s + 4, 0:512])
                for jj in range(6):
                    if g == 0:
                        pool, tag = pools6[jj]
                        state[jj] = pool.tile([128, 512], f32, tag=tag,
                                              name="wmps")
                    for cc in range(4 * g, 4 * g + 4):
                        nc.tensor.matmul(state[jj], w_sb[:, cc, jj, :],
                                         xt_t[:, cc, :],
                                         start=(cc == 0), stop=(cc == NCC - 1))
                    if g == 3:
                        dst = (qt, qt, kt, kt, vt, vt)[jj]
                        nc.scalar.copy(dst[:, jj % 2, 0:512], state[jj])
            nc.sync.dma_start(wp_sb, wproj_v)
            for hh in range(HPC):
                for tb in range(4):
                    vp = psA.tile([128, 128], f16, tag="psA")
                    nc.tensor.transpose(vp, vt[:, hh, tb * 128:(tb + 1) * 128],
                                        ident_h)
                    nc.vector.tensor_copy(v[:, tb, hh * HD:(hh + 1) * HD], vp)

        chunks = [(b, t) for b in range(B) for t in range(NTCH)]
        tiles_cur = alloc_qkv_tiles()
        tiles_nxt = None
        warmup(tiles_cur)
        ci = 1
        pending = []
        for b in range(B):
            for qg in range(NTCH):
                last = (b == B - 1 and qg == NTCH - 1)
                fillers = []
                if ci < len(chunks):
                    cb, ct = chunks[ci]
                    ci += 1
                    if cb != b:
                        tiles_nxt = alloc_qkv_tiles()
                    fillers += chunk_quanta(
                        cb, ct, tiles_cur if cb == b else tiles_nxt)
                if len(pending) >= 2:  # proj deferred two blocks: its yt
                    fillers += proj_quanta(*pending.pop(0))  # is long ready
                yt = ytp.tile([128, HPC, 512], f16, tag="yt")
                steps = unit_steps(b, qg, tiles_cur, yt, last=last)
                nf, ns = len(fillers), len(steps)
                fi = min(2, nf)
                for q in fillers[:fi]:  # prime the PE (incl. the xt DMA)
                    q()
                for si, s in enumerate(steps):
                    s()
                    tgt = min(2 + (si + 1) * (nf - 2) // ns, nf) if nf > 2 else fi
                    while fi < tgt:
                        fillers[fi]()
                        fi += 1
                while fi < nf:
                    fillers[fi]()
                    fi += 1
                pending.append((b, qg, yt))
            if tiles_nxt is not None:
                tiles_cur, tiles_nxt = tiles_nxt, None
        for pp in pending:  # (3,2) covers the last block's tail; (3,3) drains
            for q in proj_quanta(*pp):
                q()

    nc.compile()
    return nc


def _get_nc():
    if "nc" not in _CACHE:
        _CACHE["nc"] = _build_nc()
    return _CACHE["nc"]


def _make_in_maps(x2d, Wqkv, Wproj):
    hdt = np.float16
    xT = np.ascontiguousarray(x2d.T).astype(hdt)  # [C, B*T]
    in_maps = []
    for c in range(N_CORES):
        h0 = c * HPC
        cols = []
        for part in range(3):  # q, k, v blocks of Wqkv columns
            for h in range(HPC):
                j0 = part * C + (h0 + h) * HD
                cols.append(Wqkv[:, j0:j0 + HD])
        wq = np.ascontiguousarray(np.concatenate(cols, axis=1)).astype(hdt)
        wp = np.ascontiguousarray(Wproj[h0 * HD:(h0 + HPC) * HD, :]).astype(hdt)
        in_maps.append({"xt": xT, "wqkv": wq, "wproj": wp})
    return in_maps


def run_shards(in_maps, trace=False):
    from concourse.bass_utils import run_bass_kernel_spmd
    nc = _get_nc()
    last_err = None
    for _attempt in range(3):
        try:
            return run_bass_kernel_spmd(
                nc, in_maps, core_ids=list(range(N_CORES)), trace=trace)
        except Exception as e:  # transient NRT device errors — retry
            last_err = e
            if "UNAVAILABLE" not in str(e) and "UNRECOVERABLE" not in str(e):
                raise
    raise last_err


def kernel(x, Wqkv, Wproj):
    x = np.asarray(x, dtype=np.float32)
    Wqkv = np.asarray(Wqkv, dtype=np.float32)
    Wproj = np.asarray(Wproj, dtype=np.float32)
    x2d = np.ascontiguousarray(x.reshape(B * T, C))

    in_maps = _make_in_maps(x2d, Wqkv, Wproj)
    res = run_shards(in_maps)

    acc = res.results[0]["out"].astype(np.float64)
    for c in range(1, N_CORES):
        acc += res.results[c]["out"]
    return acc.reshape(B, T, C).astype(np.float32)


# revision 16
# speedup vs baseline: 1.0232x; 1.0102x over previous
"""Causal self-attention (B=4, T=2048, C=2048, H=16) on 8 trn2 NeuronCores.

Sharding: tensor-parallel over heads — 2 heads per core. Every core gets the
full (pre-transposed) activation xT, its 2 heads' slice of Wqkv columns and
Wproj rows, computes a full [B*T, C] partial output, and the host sums the 8
partials (the "all-reduce after output projection" done host-side).

v2 dataflow (all matmuls fp16 on PE; softmax tail is PE-free):
  xT tiles --DMA--> QKV proj -> Q^T,K^T [d,t] + V [t,d] (fp16)
  S^T = K^T-block.T @ Q^T chunks (PSUM f32) -> +causal mask (DVE)
  exp (ACT) -> P^T (fp16) -> DVE-accumulated denominator (acc over k-blocks)
  y^T = sum_k V_k^T-block @ P^T-block (PSUM f32)
  den all-partition sum via gpsimd.partition_all_reduce -> DVE fast reciprocal
  yt = y^T * (1/den)  (DVE) -> proj: out_partial = yt.T @ Wproj-rows -> DMA

Scheduling: the PE executes its queue in order, so the emission order IS the
schedule.  Each (b, qg) block interleaves the serial attention steps (score ->
exp -> yt, latency-bound on ACT) with independent "filler" matmul quanta: the
next t-chunk's QKV projection and the previous block's output projection.
The softmax tail (all_reduce + reciprocal + normalize) runs entirely on
gpsimd/DVE, so it never stalls the PE queue.
"""
import numpy as np

B, T, C = 4, 2048, 2048
H, HD = 16, 128
N_CORES = 8
HPC = H // N_CORES          # heads per core = 2
SCALE = float(1.0 / np.sqrt(HD))
NEG = -1e9
MM_DT = "fp16"

_CACHE = {}


def _build_nc():
    import concourse.bass as bass
    from concourse import bacc
    import concourse.tile as tile
    import concourse.mybir as mybir
    import concourse.bass_isa as bass_isa
    from concourse.masks import make_identity
    from contextlib import ExitStack

    f32 = mybir.dt.float32
    f16 = mybir.dt.float16
    Exp = mybir.ActivationFunctionType.Exp

    nc = bacc.Bacc("TRN2", target_bir_lowering=False, debug=False,
                   enable_asserts=True, num_devices=N_CORES)

    xT = nc.dram_tensor("xt", [C, B * T], f16, kind="ExternalInput").ap()
    wqkv = nc.dram_tensor("wqkv", [C, 6 * HD], f16, kind="ExternalInput").ap()
    wproj = nc.dram_tensor("wproj", [HPC * HD, C], f16, kind="ExternalInput").ap()
    out = nc.dram_tensor("out", [B * T, C], f16, kind="ExternalOutput").ap()

    wqkv_v = wqkv.rearrange("(cc p) (jj d) -> p cc jj d", p=128, d=HD)  # [128,16,6,128]
    wproj_v = wproj.rearrange("(jh p) c -> p jh c", p=128)              # [128,2,2048]
    xT_v = xT.rearrange("(cc p) t -> p cc t", p=128)                    # [128,16,B*T]

    NCC = C // 128        # 16 contraction chunks
    NTCH = T // 512       # 4 t-chunks per batch

    with tile.TileContext(nc) as tc, ExitStack() as ctx:
        const = ctx.enter_context(tc.tile_pool(name="const", bufs=1))
        wpool = ctx.enter_context(tc.tile_pool(name="w", bufs=1))
        xtp = ctx.enter_context(tc.tile_pool(name="xt", bufs=2))
        qkvp = ctx.enter_context(tc.tile_pool(name="qkv", bufs=2))
        ptp = ctx.enter_context(tc.tile_pool(name="pt", bufs=6))
        accp = ctx.enter_context(tc.tile_pool(name="acc", bufs=4))
        rp = ctx.enter_context(tc.tile_pool(name="r", bufs=6))
        ysbp = ctx.enter_context(tc.tile_pool(name="ysb", bufs=4))
        ytp = ctx.enter_context(tc.tile_pool(name="yt", bufs=3))
        op = ctx.enter_context(tc.tile_pool(name="o", bufs=6))
        psA = ctx.enter_context(tc.tile_pool(name="psA", bufs=3, space="PSUM"))
        psS = ctx.enter_context(tc.tile_pool(name="psS", bufs=3, space="PSUM"))
        psV = ctx.enter_context(tc.tile_pool(name="psV", bufs=2, space="PSUM"))

        ident_f = const.tile([128, 128], f32)
        make_identity(nc, ident_f)
        ident_h = const.tile([128, 128], f16)
        nc.scalar.copy(ident_h, ident_f)
        # transposed-orientation causal mask: keep (partition=k_rel) <= (free=q_rel)
        triT = const.tile([128, 128], f32)
        nc.gpsimd.memset(triT, 0.0)
        nc.gpsimd.affine_select(
            out=triT, in_=triT, compare_op=mybir.AluOpType.is_ge, fill=NEG,
            base=0, pattern=[[1, 128]], channel_multiplier=-1)
        ones_col = const.tile([128, 1], f16)
        nc.vector.memset(ones_col, 1.0)
        ones_row = const.tile([1, 128], f16)
        nc.vector.memset(ones_row, 1.0)

        w_sb = wpool.tile([128, NCC, 6, HD], f16)
        wp_sb = wpool.tile([128, 2, C], f16)

        def chunk_quanta(b, tch, tiles, split_dma=False):
            """QKV projection of one 512-token chunk, as a list of small
            emission quanta (fillers for the PE between attention steps)."""
            qt, kt, vt, v = tiles
            t0 = b * T + tch * 512
            xt_t = xtp.tile([128, NCC, 512], f16, tag="xt")
            quanta = []

            if split_dma:
                def dma_part(g):
                    def run():
                        nc.sync.dma_start(xt_t[:, 4 * g:4 * g + 4],
                                          xT_v[:, 4 * g:4 * g + 4, t0:t0 + 512])
                    return run
                for g in range(4):
                    quanta.append(dma_part(g))
            else:
                def dma_thunk():
                    nc.sync.dma_start(xt_t, xT_v[:, :, t0:t0 + 512])
                quanta.append(dma_thunk)

            state = {}

            def mm_thunk(jj, ccg):
                def run():
                    if ccg == 0:
                        state[jj] = psA.tile([128, 512], f32, tag="psA", name="qkps")
                    qk_ps = state[jj]
                    for cc in range(4 * ccg, 4 * ccg + 4):
                        nc.tensor.matmul(qk_ps, w_sb[:, cc, jj, :],
                                         xt_t[:, cc, :],
                                         start=(cc == 0), stop=(cc == NCC - 1))
                    if ccg == 3:
                        dst = (qt, qt, kt, kt, vt, vt)[jj]
                        nc.scalar.copy(
                            dst[:, jj % 2, tch * 512:(tch + 1) * 512], qk_ps)
                return run
            for jj in range(6):
                for ccg in range(4):
                    quanta.append(mm_thunk(jj, ccg))

            def tr_thunk(hh, tb):
                def run():
                    tg = tch * 4 + tb
                    vp = psA.tile([128, 128], f16, tag="psA")
                    nc.tensor.transpose(
                        vp, vt[:, hh, tg * 128:(tg + 1) * 128], ident_h)
                    nc.vector.tensor_copy(v[:, tg, hh * HD:(hh + 1) * HD], vp)
                return run
            for hh in range(HPC):
                for tb in range(4):
                    quanta.append(tr_thunk(hh, tb))
            return quanta

        def proj_quanta(b, qg, yt):
            quanta = []

            def pr_thunk(tt, co):
                def run():
                    o_ps = psA.tile([128, 512], f32, tag="psA")
                    for jh in range(HPC):
                        nc.tensor.matmul(
                            o_ps, yt[:, jh, tt * 128:(tt + 1) * 128],
                            wp_sb[:, jh, co * 512:(co + 1) * 512],
                            start=(jh == 0), stop=(jh == HPC - 1))
                    o_sb = op.tile([128, 512], f16, tag="osb")
                    nc.vector.tensor_copy(o_sb, o_ps)
                    r0 = b * T + qg * 512 + tt * 128
                    nc.sync.dma_start(
                        out[r0:r0 + 128, co * 512:(co + 1) * 512], o_sb)
                return run
            for tt in range(4):
                for co in range(4):
                    quanta.append(pr_thunk(tt, co))
            return quanta

        def unit_steps(b, qg, tiles, yt):
            """Attention for both heads of one 512-query group, h-interleaved.
            Softmax denominator accumulates on DVE, is column-reduced by a PE
            ones-matmul and inverted immediately (all prompt waits — nothing
            with multi-us latency ever sits at the head of an in-order
            queue).  The partition-broadcast matmul + normalize are returned
            separately ("late") to be emitted after this block's fillers, by
            which time the reciprocal is long done."""
            qt, kt, vt, v = tiles
            nkb = 4 * qg + 4
            acc = [None, None]
            yt_ps = [None, None]
            ysb = [None, None]
            rec16 = [None, None]

            def step(h, kb):
                def run():
                    kk = kb - 4 * qg
                    qs = max(0, kk) * 128
                    q0 = qg * 512
                    st = psS.tile([128, 512], f32, tag="st")
                    nc.tensor.matmul(
                        st[:, qs:512], kt[:, h, kb * 128:(kb + 1) * 128],
                        qt[:, h, q0 + qs:q0 + 512], start=True, stop=True)
                    if kk >= 0:
                        nc.vector.tensor_add(
                            st[:, qs:qs + 128], st[:, qs:qs + 128], triT)
                    pt = ptp.tile([128, 512], f16, tag="pt")
                    nc.scalar.activation(
                        pt[:, qs:512], st[:, qs:512], Exp, scale=SCALE)
                    if kb == 0:
                        acc[h] = accp.tile([128, 512], f16, tag="acc", name="accd")
                        yt_ps[h] = psV.tile([128, 512], f32, tag="psV", name="ytps")
                        nc.vector.tensor_copy(acc[h], pt)
                    else:
                        nc.vector.tensor_add(
                            acc[h][:, qs:512], acc[h][:, qs:512], pt[:, qs:512])
                    nc.tensor.matmul(
                        yt_ps[h][:, qs:512], v[:, kb, h * HD:(h + 1) * HD],
                        pt[:, qs:512], start=(kb == 0), stop=(kb == nkb - 1))
                return run

            def ycopy(h):
                def run():  # frees the psV bank without waiting on the tail
                    ysb[h] = ysbp.tile([128, 512], f32, tag="ysb", name="ysb")
                    nc.vector.tensor_copy(ysb[h], yt_ps[h])
                return run

            def den_step(h):
                def run():
                    den_ps = psA.tile([1, 512], f32, tag="psA", name="denp")
                    nc.tensor.matmul(den_ps, ones_col, acc[h],
                                     start=True, stop=True)
                    rec1 = rp.tile([1, 512], f32, tag="rec")
                    nc.vector.reciprocal_approx_fast(rec1, den_ps)
                    rec16[h] = rp.tile([1, 512], f16, tag="r", name="rec16")
                    nc.scalar.copy(rec16[h], rec1)
                return run

            def late_step(h):
                def run():
                    r_ps = psA.tile([128, 512], f32, tag="psA", name="rps")
                    nc.tensor.matmul(r_ps, ones_row, rec16[h],
                                     start=True, stop=True)
                    rsb = rp.tile([128, 512], f32, tag="rsb")
                    nc.vector.tensor_copy(rsb, r_ps)
                    nc.vector.tensor_mul(yt[:, h, :], ysb[h], rsb)
                return run

            steps = []
            for kb in range(nkb):
                for h in range(HPC):
                    steps.append(step(h, kb))
            steps.append(ycopy(0))
            steps.append(ycopy(1))
            steps.append(den_step(0))
            steps.append(den_step(1))
            return steps, [late_step(0), late_step(1)]

        def alloc_qkv_tiles():
            qt = qkvp.tile([128, HPC, T], f16, tag="qt")
            kt = qkvp.tile([128, HPC, T], f16, tag="kt")
            vt = qkvp.tile([128, HPC, T], f16, tag="vt")
            v = qkvp.tile([128, T // 128, HPC * HD], f16, tag="v")
            return (qt, kt, vt, v)

        def warmup(tiles):
            """Chunk (0,0) QKV with ccg-outer iteration so each 4-cc group of
            matmuls needs only the w/xt DMA parts already delivered.  The 6 jj
            PSUM accumulators live across ccg groups, spread over 3 pools."""
            qt, kt, vt, v = tiles
            xt_t = xtp.tile([128, NCC, 512], f16, tag="xt")
            pools6 = [(psA, "psA"), (psA, "psA"), (psS, "st"),
                      (psS, "st"), (psV, "psV"), (psV, "psV")]
            nc.sync.dma_start(w_sb[:, 0:4], wqkv_v[:, 0:4])
            nc.sync.dma_start(xt_t[:, 0:4], xT_v[:, 0:4, 0:512])
            state = {}
            for g in range(4):
                if g < 3:
                    s = 4 * (g + 1)
                    nc.sync.dma_start(w_sb[:, s:s + 4], wqkv_v[:, s:s + 4])
                    nc.sync.dma_start(xt_t[:, s:s + 4], xT_v[:, s:s + 4, 0:512])
                for jj in range(6):
                    if g == 0:
                        pool, tag = pools6[jj]
                        state[jj] = pool.tile([128, 512], f32, tag=tag,
                                              name="wmps")
                    for cc in range(4 * g, 4 * g + 4):
                        nc.tensor.matmul(state[jj], w_sb[:, cc, jj, :],
                                         xt_t[:, cc, :],
                                         start=(cc == 0), stop=(cc == NCC - 1))
                    if g == 3:
                        dst = (qt, qt, kt, kt, vt, vt)[jj]
                        nc.scalar.copy(dst[:, jj % 2, 0:512], state[jj])
            nc.sync.dma_start(wp_sb, wproj_v)
            for hh in range(HPC):
                for tb in range(4):
                    vp = psA.tile([128, 128], f16, tag="psA")
                    nc.tensor.transpose(vp, vt[:, hh, tb * 128:(tb + 1) * 128],
                                        ident_h)
                    nc.vector.tensor_copy(v[:, tb, hh * HD:(hh + 1) * HD], vp)

        chunks = [(b, t) for b in range(B) for t in range(NTCH)]
        tiles_cur = alloc_qkv_tiles()
        tiles_nxt = None
        warmup(tiles_cur)
        ci = 1
        pending = []
        for b in range(B):
            for qg in range(NTCH):
                fillers = []
                if ci < len(chunks):
                    cb, ct = chunks[ci]
                    ci += 1
                    if cb != b:
                        tiles_nxt = alloc_qkv_tiles()
                    fillers += chunk_quanta(
                        cb, ct, tiles_cur if cb == b else tiles_nxt)
                if len(pending) >= 2:  # proj deferred two blocks: its yt
                    fillers += proj_quanta(*pending.pop(0))  # is long ready
                yt = ytp.tile([128, HPC, 512], f16, tag="yt")
                steps, late = unit_steps(b, qg, tiles_cur, yt)
                nf, ns = len(fillers), len(steps)
                fi = min(2, nf)
                for q in fillers[:fi]:  # prime the PE (incl. the xt DMA)
                    q()
                for si, s in enumerate(steps):
                    s()
                    tgt = min(2 + (si + 1) * (nf - 2) // ns, nf) if nf > 2 else fi
                    while fi < tgt:
                        fillers[fi]()
                        fi += 1
                while fi < nf:
                    fillers[fi]()
                    fi += 1
                for q in late:  # broadcast+normalize after the fillers
                    q()
                pending.append((b, qg, yt))
            if tiles_nxt is not None:
                tiles_cur, tiles_nxt = tiles_nxt, None
        for pp in pending:  # (3,2) covers the last block's tail; (3,3) drains
            for q in proj_quanta(*pp):
                q()

    nc.compile()
    return nc


def _get_nc():
    if "nc" not in _CACHE:
        _CACHE["nc"] = _build_nc()
    return _CACHE["nc"]


def _make_in_maps(x2d, Wqkv, Wproj):
    hdt = np.float16
    xT = np.ascontiguousarray(x2d.T).astype(hdt)  # [C, B*T]
    in_maps = []
    for c in range(N_CORES):
        h0 = c * HPC
        cols = []
        for part in range(3):  # q, k, v blocks of Wqkv columns
            for h in range(HPC):
                j0 = part * C + (h0 + h) * HD
                cols.append(Wqkv[:, j0:j0 + HD])
        wq = np.ascontiguousarray(np.concatenate(cols, axis=1)).astype(hdt)
        wp = np.ascontiguousarray(Wproj[h0 * HD:(h0 + HPC) * HD, :]).astype(hdt)
        in_maps.append({"xt": xT, "wqkv": wq, "wproj": wp})
    return in_maps


def run_shards(in_maps, trace=False):
    from concourse.bass_utils import run_bass_kernel_spmd
    nc = _get_nc()
    last_err = None
    for _attempt in range(3):
        try:
            return run_bass_kernel_spmd(
                nc, in_maps, core_ids=list(range(N_CORES)), trace=trace)
        except Exception as e:  # transient NRT device errors — retry
            last_err = e
            if "UNAVAILABLE" not in str(e) and "UNRECOVERABLE" not in str(e):
                raise
    raise last_err


def kernel(x, Wqkv, Wproj):
    x = np.asarray(x, dtype=np.float32)
    Wqkv = np.asarray(Wqkv, dtype=np.float32)
    Wproj = np.asarray(Wproj, dtype=np.float32)
    x2d = np.ascontiguousarray(x.reshape(B * T, C))

    in_maps = _make_in_maps(x2d, Wqkv, Wproj)
    res = run_shards(in_maps)

    acc = res.results[0]["out"].astype(np.float32)
    for c in range(1, N_CORES):
        acc += res.results[c]["out"].astype(np.float32)
    return acc.reshape(B, T, C).astype(np.float32)
